# revision 1
# baseline (speedup 1.0000x reference)
"""Trainium2 Bass kernel for hyperbolic linear-attention transformer layer.

Data-parallel over nodes (N=32768) across 8 NeuronCores. Per core:
  Phase A: k/v head projections (PE, fp32r), phi_k nonlinearity (DVE/ACT),
           per-core partial ktv = phi_k^T v accumulated in PSUM, partial
           sum(phi_k) accumulated on DVE.
  AllReduce of [ktv | sumk] partials (2.1 MB) across the 8 cores.
  Phase B: q projection, phi_q, denominator folded into a per-(head,node)
           scale, attn^T computed feature-major (so the final projection
           needs no transposes), fused v_map path (W_vm = v_map_w @ Wv
           precomputed on host), final projection + Lorentz lift.

All matmuls run as float32r (full PE rate at moving-dim>=256).
"""

import os
import numpy as np
import concourse.bass as bass
import concourse.tile as tile
from concourse import bacc, mybir
from concourse.bass_utils import run_bass_kernel_spmd

F32 = mybir.dt.float32
F32R = mybir.dt.float32r
AF = mybir.ActivationFunctionType
ALU = mybir.AluOpType

NCORES = 8
N = 32768
NCHUNK = N // NCORES          # 4096 nodes per core
H = 8
D = 256
HD = H * D                    # 2048
KC = 3                        # contraction chunks: 384 = 3*128 (257 used)
EPS = 1e-6

_CACHE = {}


def _build(reps=1):
    if reps in _CACHE:
        return _CACHE[reps]
    onecore = bool(os.environ.get("KT_ONECORE"))
    nc = bacc.Bacc("TRN2", target_bir_lowering=False, debug=False,
                   num_devices=1 if onecore else NCORES)

    xqT = nc.dram_tensor("xqT", [KC, 128, NCHUNK], F32R, kind="ExternalInput").ap()
    xsT = nc.dram_tensor("xsT", [KC, 128, NCHUNK], F32R, kind="ExternalInput").ap()
    wq = nc.dram_tensor("wq", [KC, 128, HD], F32R, kind="ExternalInput").ap()
    wk = nc.dram_tensor("wk", [KC, 128, HD], F32R, kind="ExternalInput").ap()
    wv = nc.dram_tensor("wv", [KC, 128, HD], F32R, kind="ExternalInput").ap()
    wvm = nc.dram_tensor("wvm", [KC, 128, HD], F32R, kind="ExternalInput").ap()
    fw = nc.dram_tensor("fw", [16, 128, D], F32R, kind="ExternalInput").ap()
    fbias = nc.dram_tensor("fbias", [1, D], F32R, kind="ExternalInput").ap()
    ones_r = nc.dram_tensor("ones_r", [1, 128], F32R, kind="ExternalInput").ap()
    ones_c = nc.dram_tensor("ones_c", [128, 8], F32R, kind="ExternalInput").ap()
    ind = nc.dram_tensor("ind", [128, 8, 8], F32R, kind="ExternalInput").ap()
    ind2 = nc.dram_tensor("ind2", [8, 8, 128], F32R, kind="ExternalInput").ap()
    zt = nc.dram_tensor("zt", [128, 16, 8], F32R, kind="ExternalInput").ap()
    cons = nc.dram_tensor("cons", [8, 1], F32, kind="ExternalInput").ap()
    out = nc.dram_tensor("out", [NCHUNK, 257], F32, kind="ExternalOutput").ap()

    with tile.TileContext(nc) as tc:
        _body(nc, tc, reps, xqT, xsT, wq, wk, wv, wvm, fw, fbias,
              ones_r, ones_c, ind, ind2, zt, cons, out)
    nc.compile()
    _CACHE[reps] = nc
    return nc


def _body(nc, tc, reps, xqT, xsT, wq, wk, wv, wvm, fw, fbias,
          ones_r, ones_c, ind, ind2, zt, cons, out):
    import contextlib
    stack = contextlib.ExitStack()
    with stack:
        cpool = stack.enter_context(tc.tile_pool(name="const", bufs=1))
        dpool = stack.enter_context(tc.tile_pool(name="dram", bufs=1, space="DRAM"))

        ones_r_sb = cpool.tile([1, 128], F32R)
        nc.sync.dma_start(ones_r_sb[:], ones_r[:])
        ones_c_sb = cpool.tile([128, 8], F32R)
        nc.sync.dma_start(ones_c_sb[:], ones_c[:])
        ind_sb = cpool.tile([128, 8, 8], F32R)
        nc.sync.dma_start(ind_sb[:], ind[:])
        ind2_sb = cpool.tile([8, 8, 128], F32R)
        nc.sync.dma_start(ind2_sb[:], ind2[:])
        fb_sb = cpool.tile([1, D], F32R)
        nc.sync.dma_start(fb_sb[:], fbias[:])
        eps_sb = cpool.tile([8, 1], F32)
        nc.sync.dma_start(eps_sb[:], cons[:])

        ar_in = dpool.tile([129, 4096], F32)
        ar_out = dpool.tile([129, 4096], F32)

        for rep in range(reps):
            if not os.environ.get("KT_SKIP_A"):
                _phase_a(nc, tc, xsT, wk, wv, ones_c_sb, ar_in)
            if os.environ.get("KT_ONECORE"):
                nc.sync.dma_start(ar_out[:], ar_in[:])
            else:
                nc.gpsimd.collective_compute(
                    "AllReduce", ALU.add,
                    replica_groups=[list(range(NCORES))],
                    ins=[ar_in.opt()], outs=[ar_out.opt()])
            if not os.environ.get("KT_SKIP_B"):
                _phase_b(nc, tc, xqT, xsT, wq, wvm, fw, fb_sb, ones_r_sb,
                         ind_sb, ind2_sb, zt, eps_sb, ar_out, out)
            else:
                obp = tc.tile_pool(name="oBtmp", bufs=1)
                with obp as ob:
                    o_sb = ob.tile([128, 257], F32)
                    nc.sync.dma_start(o_sb[:], ar_out[0:128, 0:257])
                    for t0_ in range(NCHUNK // 128):
                        nc.sync.dma_start(out[t0_ * 128:(t0_ + 1) * 128, :], o_sb[:])


def _phase_a(nc, tc, xsT, wk, wv, ones_c_sb, ar_in):
    import contextlib
    with contextlib.ExitStack() as st:
        wpool = st.enter_context(tc.tile_pool(name="wA", bufs=1))
        xp = st.enter_context(tc.tile_pool(name="xA", bufs=3))
        zp = st.enter_context(tc.tile_pool(name="zA", bufs=2))
        yp = st.enter_context(tc.tile_pool(name="yA", bufs=2))
        scrp = st.enter_context(tc.tile_pool(name="scrA", bufs=2))
        stp = st.enter_context(tc.tile_pool(name="stA", bufs=4))
        php = st.enter_context(tc.tile_pool(name="phA", bufs=2))
        vp = st.enter_context(tc.tile_pool(name="vA", bufs=2))
        drp = st.enter_context(tc.tile_pool(name="drA", bufs=2))
        pk = st.enter_context(tc.tile_pool(name="psAk", bufs=1, space="PSUM"))
        pp = st.enter_context(tc.tile_pool(name="psAp", bufs=3, space="PSUM"))
        psk = st.enter_context(tc.tile_pool(name="psAs", bufs=1, space="PSUM"))

        wk_sb = wpool.tile([128, KC, HD], F32R)
        nc.sync.dma_start(wk_sb[:], wk.rearrange("c p n -> p c n"))
        wv_sb = wpool.tile([128, KC, HD], F32R)
        nc.sync.dma_start(wv_sb[:], wv.rearrange("c p n -> p c n"))
        sumk_acc = wpool.tile([128, HD], F32R)

        ntiles = int(os.environ.get("KT_NTILES", NCHUNK // 128))
        for g in range(2):
            gofs = g * 1024
            ktv_ps = pk.tile([128, 4, 512], F32)
            for t in range(ntiles):
                xs_sb = xp.tile([128, KC, 128], F32R, tag="xs")
                nc.sync.dma_start(
                    xs_sb[:],
                    xsT[:, :, t * 128:(t + 1) * 128].rearrange("c p n -> p c n"))

                ks_ps = []
                vs_ps = []
                for blk in range(2):
                    kp_t = pp.tile([128, 512], F32, tag="projA")
                    for c in range(KC):
                        nc.tensor.matmul(
                            kp_t[:], lhsT=xs_sb[:, c],
                            rhs=wk_sb[:, c, gofs + blk * 512: gofs + blk * 512 + 512],
                            start=(c == 0), stop=(c == KC - 1))
                    ks_ps.append(kp_t)
                for blk in range(2):
                    vp_t = pp.tile([128, 512], F32, tag="projA")
                    for c in range(KC):
                        nc.tensor.matmul(
                            vp_t[:], lhsT=xs_sb[:, c],
                            rhs=wv_sb[:, c, gofs + blk * 512: gofs + blk * 512 + 512],
                            start=(c == 0), stop=(c == KC - 1))
                    vs_ps.append(vp_t)

                # z = relu(ks) + eps
                z = zp.tile([128, 1024], F32, tag="z")
                for blk in range(2):
                    nc.vector.tensor_scalar(
                        z[:, blk * 512:(blk + 1) * 512], ks_ps[blk][:],
                        0.0, EPS, ALU.max, ALU.add)
                # v copy to SBUF (frees psum quickly)
                v_sb = vp.tile([128, 1024], F32R, tag="v")
                for blk in range(2):
                    nc.scalar.copy(v_sb[:, blk * 512:(blk + 1) * 512], vs_ps[blk][:])

                # y = z^2 with per-head accumulated sums
                y = yp.tile([128, 1024], F32R, tag="y")
                sy = stp.tile([128, 4], F32, tag="sy")
                sy2 = stp.tile([128, 4], F32, tag="sy2")
                for hh in range(4):
                    sl = slice(hh * 256, hh * 256 + 256)
                    nc.scalar.activation(y[:, sl], z[:, sl], AF.Square,
                                         accum_out=sy[:, hh:hh + 1])
                for hh in range(4):
                    sl = slice(hh * 256, hh * 256 + 256)
                    scr = scrp.tile([128, 256], F32, tag="y2scr")
                    nc.scalar.activation(scr[:], y[:, sl].bitcast(F32), AF.Square,
                                         accum_out=sy2[:, hh:hh + 1])
                # factor = sqrt(sy / sy2)
                rec = stp.tile([128, 4], F32, tag="rec")
                nc.vector.reciprocal(rec[:], sy2[:])
                rat = stp.tile([128, 4], F32, tag="rat")
                nc.vector.tensor_mul(rat[:], sy[:], rec[:])
                fac = stp.tile([128, 4], F32, tag="fac")
                nc.scalar.activation(fac[:], rat[:], AF.Sqrt)

                phi = php.tile([128, 1024], F32R, tag="phi")
                for hh in range(4):
                    sl = slice(hh * 256, hh * 256 + 256)
                    nc.vector.tensor_scalar_mul(phi[:, sl], y[:, sl].bitcast(F32),
                                                fac[:, hh:hh + 1])
                # sumk accumulation
                dst = sumk_acc[:, gofs:gofs + 1024]
                if t == 0:
                    nc.scalar.copy(dst, phi[:].bitcast(F32))
                else:
                    nc.vector.tensor_add(dst, dst.bitcast(F32), phi[:].bitcast(F32))

                # ktv accumulation: ktv[h][m,d] += phi[:,h*256+mc*128]T v[:,h*256:]
                for hh in range(4 if not os.environ.get("KT_NO_KTV") else 0):
                    for mc in range(2):
                        nc.tensor.matmul(
                            ktv_ps[:, hh, mc * 256: mc * 256 + 256],
                            lhsT=phi[:, hh * 256 + mc * 128: hh * 256 + mc * 128 + 128],
                            rhs=v_sb[:, hh * 256: hh * 256 + 256],
                            start=(t == 0), stop=(t == ntiles - 1))

            # drain ktv partials for this head group
            if not os.environ.get("KT_NO_KTV"):
                ktv_sbt = drp.tile([128, 4, 512], F32, tag="ktvdr")
                for hh in range(4):
                    nc.scalar.copy(ktv_sbt[:, hh], ktv_ps[:, hh])
                nc.sync.dma_start(ar_in[0:128, g * 2048:(g + 1) * 2048],
                                  ktv_sbt[:].rearrange("p a b -> p (a b)"))
            # sumk partition-reduction for this group
            for blk in range(2 if not os.environ.get("KT_NO_SUMK") else 0):
                sps = psk.tile([8, 512], F32, tag="sumkps")
                nc.tensor.matmul(
                    sps[:], lhsT=ones_c_sb[:],
                    rhs=sumk_acc[:, gofs + blk * 512: gofs + blk * 512 + 512],
                    start=True, stop=True)
                srow = drp.tile([1, 512], F32, tag="srow")
                nc.scalar.copy(srow[:], sps[0:1, :])
                nc.sync.dma_start(
                    ar_in[128:129, gofs + blk * 512: gofs + blk * 512 + 512],
                    srow[:])


def _phase_b(nc, tc, xqT, xsT, wq, wvm, fw, fb_sb, ones_r_sb, ind_sb, ind2_sb,
             zt, eps_sb, ar_out, out):
    import contextlib
    with contextlib.ExitStack() as st:
        wpool = st.enter_context(tc.tile_pool(name="wB", bufs=1))
        xp = st.enter_context(tc.tile_pool(name="xB", bufs=2))
        zp = st.enter_context(tc.tile_pool(name="zB", bufs=3))
        yp = st.enter_context(tc.tile_pool(name="yB", bufs=17))
        y2p = st.enter_context(tc.tile_pool(name="y2B", bufs=3))
        stp = st.enter_context(tc.tile_pool(name="stB", bufs=2))
        php = st.enter_context(tc.tile_pool(name="phB", bufs=17))
        atp = st.enter_context(tc.tile_pool(name="atB", bufs=17))
        obp = st.enter_context(tc.tile_pool(name="oB", bufs=3))
        qp = st.enter_context(tc.tile_pool(name="psBq", bufs=2, space="PSUM"))
        sump = st.enter_context(tc.tile_pool(name="psBs", bufs=1, space="PSUM"))
        sbp = st.enter_context(tc.tile_pool(name="psBb", bufs=1, space="PSUM"))
        ap_ = st.enter_context(tc.tile_pool(name="psBa", bufs=2, space="PSUM"))
        op = st.enter_context(tc.tile_pool(name="psBo", bufs=1, space="PSUM"))

        wq_sb = wpool.tile([128, KC, HD], F32R)
        nc.sync.dma_start(wq_sb[:], wq.rearrange("c p n -> p c n"))
        wvm_sb = wpool.tile([128, KC, HD], F32R)
        nc.sync.dma_start(wvm_sb[:], wvm.rearrange("c p n -> p c n"))
        fw_sb = wpool.tile([128, 16, D], F32R)
        nc.sync.dma_start(fw_sb[:], fw.rearrange("c p n -> p c n"))
        # ktv (all-reduced) as lhsT chunks [m_loc, h, mc, dc, d_loc]
        ktv_sb = wpool.tile([128, H, 2, 2, 128], F32R)
        nc.gpsimd.dma_start(
            ktv_sb[:],
            ar_out[0:128, :].rearrange("p (h mc dc dl) -> p h mc dc dl",
                                       h=H, mc=2, dc=2))
        # sumk chunk columns: [128, 16, 8], chunk c -> column h(c)
        sumk_w = wpool.tile([128, 16, 8], F32R)
        nc.sync.dma_start(sumk_w[:], zt[:])
        for c in range(16):
            hh = c // 2
            nc.gpsimd.dma_start(
                sumk_w[:, c, hh:hh + 1],
                ar_out[128:129, c * 128:(c + 1) * 128].rearrange(
                    "r (p o) -> (r p) o", o=1))

        NST = 256                      # supertile node count
        nst = int(os.environ.get("KT_NST", NCHUNK // NST))
        for stx in range(nst):
            nofs = stx * NST
            xq_sb = xp.tile([128, KC, NST], F32R, tag="xq")
            nc.sync.dma_start(
                xq_sb[:], xqT[:, :, nofs:nofs + NST].rearrange("c p n -> p c n"))
            xs_sb = xp.tile([128, KC, NST], F32R, tag="xsB")
            nc.sync.dma_start(
                xs_sb[:], xsT[:, :, nofs:nofs + NST].rearrange("c p n -> p c n"))

            sums_ps = sump.tile([8, 3, NST], F32, tag="sums")
            ys = []
            for c in range(16):
                hh = c // 2
                q_ps = qp.tile([128, NST], F32, tag="qps")
                for kc in range(KC):
                    nc.tensor.matmul(
                        q_ps[:], lhsT=wq_sb[:, kc, c * 128:(c + 1) * 128],
                        rhs=xq_sb[:, kc], start=(kc == 0), stop=(kc == KC - 1))
                z = zp.tile([128, NST], F32, tag="zB")
                nc.vector.tensor_scalar(z[:], q_ps[:], 0.0, EPS, ALU.max, ALU.add)
                y_c = yp.tile([128, NST], F32R, tag="yB")
                nc.scalar.activation(y_c[:], z[:], AF.Square)
                y2 = y2p.tile([128, NST], F32R, tag="y2B")
                nc.scalar.activation(y2[:], y_c[:].bitcast(F32), AF.Square)
                nc.tensor.matmul(sums_ps[:, 0], lhsT=ind_sb[:, hh], rhs=y_c[:],
                                 start=(c == 0), stop=(c == 15))
                nc.tensor.matmul(sums_ps[:, 1], lhsT=ind_sb[:, hh], rhs=y2[:],
                                 start=(c == 0), stop=(c == 15))
                nc.tensor.matmul(sums_ps[:, 2], lhsT=sumk_w[:, c], rhs=y_c[:],
                                 start=(c == 0), stop=(c == 15))
                ys.append(y_c)

            # stats on [8, NST]
            rec2 = stp.tile([8, NST], F32, tag="rec2")
            nc.vector.reciprocal(rec2[:], sums_ps[:, 1])
            rat = stp.tile([8, NST], F32, tag="ratB")
            nc.vector.tensor_mul(rat[:], sums_ps[:, 0], rec2[:])
            fac = stp.tile([8, NST], F32, tag="facB")
            nc.scalar.activation(fac[:], rat[:], AF.Sqrt)
            den = stp.tile([8, NST], F32, tag="den")
            nc.vector.tensor_mul(den[:], sums_ps[:, 2], fac[:])
            nc.vector.tensor_scalar_add(den[:], den[:], eps_sb[:])
            rden = stp.tile([8, NST], F32, tag="rden")
            nc.vector.reciprocal(rden[:], den[:])
            s_sb = stp.tile([8, NST], F32R, tag="sB")
            nc.vector.tensor_mul(s_sb[:], fac[:], rden[:])

            # phi' = y * s (s broadcast across partitions via K=1 matmul)
            phis = []
            for hh in range(8):
                sbc = sbp.tile([128, NST], F32, tag="sbc")
                nc.tensor.matmul(sbc[:], lhsT=ind2_sb[:, hh], rhs=s_sb[:],
                                 start=True, stop=True)
                for mc in range(2):
                    phi_c = php.tile([128, NST], F32R, tag="phB")
                    nc.vector.tensor_mul(phi_c[:], ys[2 * hh + mc][:].bitcast(F32),
                                         sbc[:])
                    phis.append(phi_c)

            # attnT chunks: attnT[(h,dc)] = sum_mc ktv[h,mc,dc]^T phi[(h,mc)] + vssT
            ats = []
            for c in range(16):
                hh, dc = c // 2, c % 2
                at_ps = ap_.tile([128, NST], F32, tag="atps")
                for mc in range(2):
                    nc.tensor.matmul(at_ps[:], lhsT=ktv_sb[:, hh, mc, dc],
                                     rhs=phis[2 * hh + mc][:],
                                     start=(mc == 0), stop=False)
                for kc in range(KC):
                    nc.tensor.matmul(at_ps[:], lhsT=wvm_sb[:, kc, c * 128:(c + 1) * 128],
                                     rhs=xs_sb[:, kc],
                                     start=False, stop=(kc == KC - 1))
                at_sb = atp.tile([128, NST], F32R, tag="atB")
                nc.scalar.copy(at_sb[:], at_ps[:])
                ats.append(at_sb)

            # final projection per 128-node subtile + Lorentz lift
            for sn in range(NST // 128):
                o_ps = op.tile([128, D], F32, tag="ops")
                for c in range(16):
                    nc.tensor.matmul(o_ps[:], lhsT=ats[c][:, sn * 128:(sn + 1) * 128],
                                     rhs=fw_sb[:, c], start=(c == 0), stop=False)
                nc.tensor.matmul(o_ps[:], lhsT=ones_r_sb[:], rhs=fb_sb[:],
                                 start=False, stop=True)
                sq = zp.tile([128, D], F32, tag="sqB")
                ssum = stp.tile([128, 1], F32, tag="ssum")
                nc.scalar.activation(sq[:], o_ps[:], AF.Square,
                                     accum_out=ssum[:])
                tcol = stp.tile([128, 1], F32, tag="tcol")
                nc.scalar.activation(tcol[:], ssum[:], AF.Sqrt, bias=1.0)
                o_sb = obp.tile([128, 257], F32, tag="osb")
                nc.vector.tensor_copy(o_sb[:, 1:257], o_ps[:])
                nc.vector.tensor_copy(o_sb[:, 0:1], tcol[:])
                nc.sync.dma_start(out[nofs + sn * 128: nofs + (sn + 1) * 128, :],
                                  o_sb[:])


def _prep_inputs(query_input, source_input, Wq_w, Wq_b, Wk_w, Wk_b, Wv_w, Wv_b,
                 norm_scale, v_map_w, v_map_b, final_w, final_b):
    def pad_x(x):
        xt = np.zeros((KC * 128, N), np.float32)
        xt[0:257] = x.T
        xt[257] = 1.0
        return xt.reshape(KC, 128, N)

    def pad_w(w_flat, b_flat):
        wt = np.zeros((KC * 128, HD), np.float32)
        wt[0:257] = w_flat.T
        wt[257] = b_flat
        return wt.reshape(KC, 128, HD)

    xq = pad_x(np.asarray(query_input))
    xs = pad_x(np.asarray(source_input))
    wq_h = pad_w(np.asarray(Wq_w).reshape(HD, 257), np.asarray(Wq_b).reshape(HD))
    wk_h = pad_w(np.asarray(Wk_w).reshape(HD, 257), np.asarray(Wk_b).reshape(HD))
    wv_h = pad_w(np.asarray(Wv_w).reshape(HD, 257), np.asarray(Wv_b).reshape(HD))

    vm = np.asarray(v_map_w)
    # wvm_flat[h] = vm @ Wv_w[h]  -> [H, 256, 257]
    wvm_flat = np.einsum('od,hdi->hoi', vm, np.asarray(Wv_w))
    bvm = (np.asarray(Wv_b) @ vm.T + np.asarray(v_map_b)[None, :]).reshape(HD)
    wvm_h = pad_w(wvm_flat.reshape(HD, 257), bvm)

    fw_h = np.ascontiguousarray(np.asarray(final_w).T).reshape(16, 128, D)
    fb_h = np.asarray(final_b).reshape(1, D).astype(np.float32)

    s = abs(float(np.asarray(norm_scale))) + EPS
    eps_eff = EPS * s * s
    cons = np.full((8, 1), eps_eff, np.float32)

    ind = np.zeros((128, 8, 8), np.float32)
    for hh in range(8):
        ind[:, hh, hh] = 1.0
    ind2 = np.zeros((8, 8, 128), np.float32)
    for hh in range(8):
        ind2[hh, hh, :] = 1.0

    common = {
        "wq": wq_h, "wk": wk_h, "wv": wv_h, "wvm": wvm_h,
        "fw": fw_h.astype(np.float32), "fbias": fb_h,
        "ones_r": np.ones((1, 128), np.float32),
        "ones_c": np.ones((128, 8), np.float32),
        "ind": ind, "ind2": ind2, "zt": np.zeros((128, 16, 8), np.float32),
        "cons": cons,
    }
    in_maps = []
    for c in range(NCORES):
        m = dict(common)
        m["xqT"] = np.ascontiguousarray(xq[:, :, c * NCHUNK:(c + 1) * NCHUNK])
        m["xsT"] = np.ascontiguousarray(xs[:, :, c * NCHUNK:(c + 1) * NCHUNK])
        in_maps.append(m)
    return in_maps


def kernel(reps=1, **inputs):
    nc = _build(reps)
    in_maps = _prep_inputs(**inputs)
    res = run_bass_kernel_spmd(nc, in_maps, list(range(NCORES)))
    return np.concatenate([res.results[c]["out"] for c in range(NCORES)], axis=0)



# revision 32
# speedup vs baseline: 5317.6034x; 5317.6034x over previous
"""Trainium2 Bass kernel for hyperbolic linear-attention transformer layer.

Data-parallel over nodes (N=32768) across 8 NeuronCores.

Math (per rep):
  Phase A (source nodes, node-major tiles of 128):
    k = Wk x_pad  (PE, fp32r) ; z = relu(k) (ACT)
    y = z^2 with per-head sums sy (DVE tensor_tensor_reduce)
    sy2 = sum(y^2) per head (ACT Square accum / DVE ttr)
    phi_k = y * sqrt(sy/sy2)   (in-place)
    B += x_pad^T phi_k         (PE, PSUM accumulation over node tiles)
      - x_pad has a trailing 1-column, so row 257 of B is sum_n(phi_k) "sumk"
  AllReduce(B) over the 8 cores  (2.1 MB, Shared output).
  Mid: G[h] = B_h^T WG_h where WG_h = Wv_pad_h fw_h^T (host-precomputed);
    this folds ktv = B^T Wv and the final projection fw into one matrix.
    sumk columns are DMA-staged into the stats lhsT (zt2).
  Phase B (query nodes, feature-major supertiles of 512):
    q = Wq x_pad (PE) ; z = relu(q) ; y = z^2 ; y2 = y^2
    A,C sums via one matmul per 128-feat chunk (lhsT = [ind | sumk]),
    Bsum via matmul vs y2.  fac = sqrt(A/Bsum); s = fac/(C*fac + eps).
    phi' = y * s (broadcast via K=8 matmul)
    outT = sum_h G_h^T phi'_h + W2 x_s  (W2 = fw @ (vmap Wv) folded on host,
      bias row included)
    PE-transpose outT -> node-major, Lorentz lift, DMA out.

All matmuls are fp32r with moving dim >= 256 (full PE rate).
"""

import os
import numpy as np
import concourse.bass as bass
import concourse.tile as tile
from concourse import bacc, mybir
from concourse.bass_utils import run_bass_kernel_spmd

F32 = mybir.dt.float32
F32R = mybir.dt.float32r
AF = mybir.ActivationFunctionType
ALU = mybir.AluOpType

NCORES = 8
N = 32768
NCHUNK = N // NCORES          # 4096 nodes per core
H = 8
D = 256
HD = H * D                    # 2048
KC = 3                        # contraction chunks: 384 = 3*128 (258 used)
EPS = 1e-6
NST = 512                     # phase-B supertile node count

_CACHE = {}


def _build(reps=1):
    if reps in _CACHE:
        return _CACHE[reps]
    nc = bacc.Bacc("TRN2", target_bir_lowering=False, debug=False,
                   num_devices=NCORES)

    xqT = nc.dram_tensor("xqT", [KC, 128, NCHUNK], F32R, kind="ExternalInput").ap()
    xsT = nc.dram_tensor("xsT", [KC, 128, NCHUNK], F32R, kind="ExternalInput").ap()
    xnm = nc.dram_tensor("xnm", [NCHUNK, KC * 128], F32R, kind="ExternalInput").ap()
    wq = nc.dram_tensor("wq", [KC, 128, HD], F32R, kind="ExternalInput").ap()
    wk = nc.dram_tensor("wk", [KC, 128, HD], F32R, kind="ExternalInput").ap()
    w2 = nc.dram_tensor("w2", [KC, 128, D], F32R, kind="ExternalInput").ap()
    wg = nc.dram_tensor("wg", [KC, 128, H, D], F32R, kind="ExternalInput").ap()
    zt2 = nc.dram_tensor("zt2", [128, 16, 40], F32R, kind="ExternalInput").ap()
    ind = nc.dram_tensor("ind", [128, 8, 8], F32R, kind="ExternalInput").ap()
    ind2 = nc.dram_tensor("ind2", [8, 8, 128], F32R, kind="ExternalInput").ap()
    ident = nc.dram_tensor("ident", [128, 128], F32R, kind="ExternalInput").ap()
    cons = nc.dram_tensor("cons", [8, 1], F32, kind="ExternalInput").ap()
    out = nc.dram_tensor("out", [NCHUNK, 257], F32, kind="ExternalOutput").ap()

    with tile.TileContext(nc) as tc:
        _body(nc, tc, reps, xqT, xsT, xnm, wq, wk, w2, wg, zt2, ind, ind2,
              ident, cons, out)
    nc.compile()
    _CACHE[reps] = nc
    return nc


def _body(nc, tc, reps, xqT, xsT, xnm, wq, wk, w2, wg, zt2, ind, ind2,
          ident, cons, out):
    import contextlib
    stack = contextlib.ExitStack()
    with stack:
        cpool = stack.enter_context(tc.tile_pool(name="const", bufs=1))
        dpool = stack.enter_context(tc.tile_pool(name="dram", bufs=1, space="DRAM"))

        ind_sb = cpool.tile([128, 8, 8], F32R)
        nc.sync.dma_start(ind_sb[:], ind[:])
        ind2_sb = cpool.tile([8, 8, 128], F32R)
        nc.sync.dma_start(ind2_sb[:], ind2[:])
        ident_sb = cpool.tile([128, 128], F32R)
        nc.sync.dma_start(ident_sb[:], ident[:])
        eps_sb = cpool.tile([8, 1], F32)
        nc.sync.dma_start(eps_sb[:], cons[:])
        zt_sb = cpool.tile([128, 16, 40], F32R)
        nc.sync.dma_start(zt_sb[:], zt2[:])

        for rep in range(reps):
            ar_in = [dpool.tile([258, 1024], F32, tag=f"ari{rep}g{g}",
                                name=f"ari{rep}g{g}") for g in range(2)]
            ar_out = [dpool.tile([258, 1024], F32,
                                 tag=f"aro{rep}g{g}", name=f"aro{rep}g{g}")
                      for g in range(2)]
            if not os.environ.get("KT_SKIP_A"):
                _phase_a(nc, tc, xsT, xnm, wk, ar_in, ar_out)
            if not os.environ.get("KT_SKIP_B"):
                _phase_b(nc, tc, xqT, xsT, wq, w2, wg, zt_sb, ind_sb, ind2_sb,
                         ident_sb, eps_sb, ar_out, out)
            else:
                with tc.tile_pool(name="oBtmp", bufs=1) as ob:
                    o_sb = ob.tile([128, 257], F32)
                    nc.sync.dma_start(o_sb[:], ar_out[0][0:128, 0:257])
                    for t0_ in range(NCHUNK // 128):
                        nc.sync.dma_start(out[t0_ * 128:(t0_ + 1) * 128, :], o_sb[:])


def _phase_a(nc, tc, xsT, xnm, wk, ar_in, ar_out):
    import contextlib
    with contextlib.ExitStack() as st:
        wpool = st.enter_context(tc.tile_pool(name="wA", bufs=1))
        xp = st.enter_context(tc.tile_pool(name="xA", bufs=3))
        xnp_ = st.enter_context(tc.tile_pool(name="xnA", bufs=4))
        yp = st.enter_context(tc.tile_pool(name="yA", bufs=3))
        zap = st.enter_context(tc.tile_pool(name="zA", bufs=2))
        scp = st.enter_context(tc.tile_pool(name="scA", bufs=2))
        stp = st.enter_context(tc.tile_pool(name="stA", bufs=4))
        drp = st.enter_context(tc.tile_pool(name="drA", bufs=2))
        pk = st.enter_context(tc.tile_pool(name="psAk", bufs=2, space="PSUM"))
        pb = st.enter_context(tc.tile_pool(name="psAb", bufs=1, space="PSUM"))

        wk_sb = wpool.tile([128, KC, HD], F32R)
        nc.sync.dma_start(wk_sb[:], wk.rearrange("c p n -> p c n"))

        ntiles = NCHUNK // 128
        for g in range(2):
            gofs = g * 1024
            b_ps0 = pb.tile([128, 1024], F32, tag="bps0")
            b_ps1 = pb.tile([128, 1024], F32, tag="bps1")
            b_ps2 = pb.tile([2, 1024], F32, tag="bps2")
            b_tiles = (b_ps0, b_ps1, b_ps2)
            prev = None
            for t in range(ntiles):
                xs_sb = xp.tile([128, KC, 128], F32R, tag="xs")
                nc.sync.dma_start(
                    xs_sb[:],
                    xsT[:, :, t * 128:(t + 1) * 128].rearrange("c p n -> p c n"))
                xn_sb = xnp_.tile([128, KC, 128], F32R, tag="xn")
                nc.sync.dma_start(
                    xn_sb[:],
                    xnm[t * 128:(t + 1) * 128, :].rearrange("n (c f) -> n c f",
                                                            c=KC))

                y = yp.tile([128, 1024], F32R, tag="y")
                yf = y.bitcast(F32)
                z = zap.tile([128, 1024], F32, tag="zA")
                sy = stp.tile([128, 4], F32, tag="sy")
                sy2 = stp.tile([128, 4], F32, tag="sy2")
                for blk in range(2):
                    kp = pk.tile([128, 512], F32, tag="kp")
                    for c in range(KC):
                        nc.tensor.matmul(
                            kp[:], lhsT=xs_sb[:, c],
                            rhs=wk_sb[:, c, gofs + blk * 512: gofs + blk * 512 + 512],
                            start=(c == 0), stop=(c == KC - 1))
                    nc.vector.tensor_scalar_max(
                        z[:, blk * 512:(blk + 1) * 512], kp[:], 0.0)
                # y = z^2 with per-head accumulated sums sy (ACT Square+accum)
                for hh in range(4):
                    sl = slice(hh * 256, hh * 256 + 256)
                    nc.scalar.activation(y[:, sl], z[:, sl], AF.Square,
                                         accum_out=sy[:, hh:hh + 1])
                # sy2 = sum(y^2) per head (ACT Square with accum)
                for hh in range(4):
                    sl = slice(hh * 256, hh * 256 + 256)
                    scr = scp.tile([128, 256], F32, tag="scr")
                    nc.scalar.activation(scr[:], yf[:, sl], AF.Square,
                                         accum_out=sy2[:, hh:hh + 1])
                rec = stp.tile([128, 4], F32, tag="rec")
                nc.vector.reciprocal(rec[:], sy2[:])
                rat = stp.tile([128, 4], F32, tag="rat")
                nc.vector.tensor_mul(rat[:], sy[:], rec[:])
                fac = stp.tile([128, 4], F32, tag="fac")
                nc.scalar.activation(fac[:], rat[:], AF.Sqrt)
                # phi = y * fac, in place (2 on DVE, 2 on ACT)
                for hh in range(4):
                    sl = slice(hh * 256, hh * 256 + 256)
                    if hh < 2:
                        nc.vector.tensor_scalar_mul(y[:, sl], yf[:, sl],
                                                    fac[:, hh:hh + 1])
                    else:
                        nc.scalar.activation(y[:, sl], yf[:, sl], AF.Copy,
                                             scale=fac[:, hh:hh + 1])

                # B accumulation for the PREVIOUS tile (software pipeline:
                # keeps PE busy with tile t's projection while tile t-1's
                # phi chain drains on DVE/ACT)
                if t > 0:
                    _b_accum(nc, b_tiles, prev[0], prev[1], t - 1, ntiles)
                prev = (xn_sb, y)
            _b_accum(nc, b_tiles, prev[0], prev[1], ntiles - 1, ntiles)

            d0 = drp.tile([128, 1024], F32, tag="d0")
            nc.vector.tensor_copy(d0[:], b_ps0[:])
            nc.sync.dma_start(ar_in[g][0:128, :], d0[:])
            d1 = drp.tile([128, 1024], F32, tag="d1")
            nc.scalar.copy(d1[:], b_ps1[:])
            nc.sync.dma_start(ar_in[g][128:256, :], d1[:])
            d2 = drp.tile([2, 1024], F32, tag="d2")
            nc.vector.tensor_copy(d2[:], b_ps2[:])
            nc.sync.dma_start(ar_in[g][256:258, :], d2[:])
            # per-group AllReduce: group 0's AR overlaps group 1's compute
            nc.gpsimd.collective_compute(
                "AllReduce", ALU.add,
                replica_groups=[list(range(NCORES))],
                ins=[ar_in[g].opt()], outs=[ar_out[g].opt()])


def _b_accum(nc, b_tiles, xn_sb, y, t, ntiles):
    b_ps0, b_ps1, b_ps2 = b_tiles
    for blk in range(2):
        ms = slice(blk * 512, blk * 512 + 512)
        nc.tensor.matmul(b_ps0[:, ms], lhsT=xn_sb[:, 0], rhs=y[:, ms],
                         start=(t == 0), stop=(t == ntiles - 1))
        nc.tensor.matmul(b_ps1[:, ms], lhsT=xn_sb[:, 1], rhs=y[:, ms],
                         start=(t == 0), stop=(t == ntiles - 1))
        nc.tensor.matmul(b_ps2[:, ms], lhsT=xn_sb[:, 2, 0:2], rhs=y[:, ms],
                         start=(t == 0), stop=(t == ntiles - 1))


def _phase_b(nc, tc, xqT, xsT, wq, w2, wg, zt_sb, ind_sb, ind2_sb,
             ident_sb, eps_sb, ar_out, out):
    import contextlib
    with contextlib.ExitStack() as st:
        wpool = st.enter_context(tc.tile_pool(name="wB", bufs=1))
        mpool = st.enter_context(tc.tile_pool(name="midB", bufs=1))
        xp = st.enter_context(tc.tile_pool(name="xB", bufs=2))
        yp = st.enter_context(tc.tile_pool(name="yB", bufs=30))
        zbp = st.enter_context(tc.tile_pool(name="zbB", bufs=3))
        y2p = st.enter_context(tc.tile_pool(name="y2B", bufs=3))
        scp2 = st.enter_context(tc.tile_pool(name="sc2B", bufs=2))
        stp = st.enter_context(tc.tile_pool(name="stB", bufs=1))
        sbp = st.enter_context(tc.tile_pool(name="sbB", bufs=2))
        asb = st.enter_context(tc.tile_pool(name="aB", bufs=3))
        obp = st.enter_context(tc.tile_pool(name="oB", bufs=3))
        ps = st.enter_context(tc.tile_pool(name="psBs", bufs=2, space="PSUM"))
        pbs = st.enter_context(tc.tile_pool(name="psBbs", bufs=2, space="PSUM"))
        pa = st.enter_context(tc.tile_pool(name="psBa", bufs=2, space="PSUM"))
        pst = st.enter_context(tc.tile_pool(name="psBt", bufs=2, space="PSUM"))

        wq_sb = wpool.tile([128, KC, HD], F32R)
        nc.sync.dma_start(wq_sb[:], wq.rearrange("c p n -> p c n"))
        w2_sb = wpool.tile([128, KC, D], F32R)
        nc.sync.dma_start(w2_sb[:], w2.rearrange("c p n -> p c n"))
        g_sb = wpool.tile([128, H, 2, D], F32R)

        def mid_g(g):
            # G = B^T WG for this half's heads; stage sumk cols into zt
            bf0 = mpool.tile([128, 1024], F32R, tag="bf0")
            nc.sync.dma_start(bf0.bitcast(F32)[:], ar_out[g][0:128, :])
            bf1 = mpool.tile([128, 1024], F32R, tag="bf1")
            nc.sync.dma_start(bf1.bitcast(F32)[:], ar_out[g][128:256, :])
            bf2 = mpool.tile([2, 1024], F32R, tag="bf2")
            nc.sync.dma_start(bf2.bitcast(F32)[:], ar_out[g][256:258, :])
            wg_sb = mpool.tile([128, KC, 4, D], F32R, tag="wgh")
            nc.sync.dma_start(
                wg_sb[:], wg[:, :, g * 4:(g + 1) * 4].rearrange(
                    "c p h n -> p c h n"))
            bfs = [bf0, bf1, bf2]
            for hl in range(4):
                hh = g * 4 + hl
                for mc in range(2):
                    msl = slice(hl * 256 + mc * 128, hl * 256 + mc * 128 + 128)
                    gp = pa.tile([128, NST], F32, tag="mm")
                    for fc in range(KC):
                        lhs = bfs[fc][:, msl] if fc < 2 else bfs[2][0:2, msl]
                        rhs = (wg_sb[:, fc, hl] if fc < 2
                               else wg_sb[0:2, fc, hl])
                        nc.tensor.matmul(gp[:, 0:D], lhsT=lhs, rhs=rhs,
                                         start=(fc == 0), stop=(fc == KC - 1))
                    if (hl * 2 + mc) % 2 == 0:
                        nc.vector.tensor_copy(g_sb[:, hh, mc], gp[:, 0:D])
                    else:
                        nc.scalar.copy(g_sb[:, hh, mc], gp[:, 0:D])
            for cl in range(8):
                c = g * 8 + cl
                hh = c // 2
                nc.gpsimd.dma_start(
                    zt_sb[:, c, 32 + hh:33 + hh],
                    ar_out[g][257:258, cl * 128:(cl + 1) * 128].rearrange(
                        "r (p o) -> (r p) o", o=1))

        state = {}

        def pre(stx):
            nofs = stx * NST
            xq_sb = xp.tile([128, KC, NST], F32R, tag="xq")
            nc.sync.dma_start(
                xq_sb[:], xqT[:, :, nofs:nofs + NST].rearrange("c p n -> p c n"))
            sums_ps = ps.tile([64, NST], F32, tag="sums")
            bs_ps = pbs.tile([8, NST], F32, tag="bsum")
            ys = []
            for c in range(16):
                qp = pa.tile([128, NST], F32, tag="mm")
                for kc in range(KC):
                    nc.tensor.matmul(
                        qp[:], lhsT=wq_sb[:, kc, c * 128:(c + 1) * 128],
                        rhs=xq_sb[:, kc], start=(kc == 0), stop=(kc == KC - 1))
                zb = zbp.tile([128, NST], F32, tag="zbB")
                nc.vector.tensor_scalar_max(zb[:], qp[:], 0.0)
                y = yp.tile([128, NST], F32R, tag="yB")
                yf = y.bitcast(F32)
                nc.scalar.activation(y[:], zb[:], AF.Square)
                y2 = y2p.tile([128, NST], F32R, tag="y2B")
                nc.scalar.activation(y2[:], yf[:], AF.Square)
                nc.tensor.matmul(bs_ps[:], lhsT=ind_sb[:, c // 2],
                                 rhs=y2[:], start=(c == 0), stop=(c == 15))
                ys.append(y)
            state[stx] = (ys, sums_ps, bs_ps)

        def post(stx):
            nofs = stx * NST
            ys, sums_ps, bs_ps = state.pop(stx)
            # A,C = per-head sums of y and sumk-weighted sums of y
            # (lhsT cols 0..7 = head indicators, 8..15 = sumk staged from AR)
            for c in range(16):
                nc.tensor.matmul(sums_ps[0:40], lhsT=zt_sb[:, c], rhs=ys[c][:],
                                 start=(c == 0), stop=(c == 15))
            # stats: fac = sqrt(A/Bsum); s = fac / (C*fac + eps)
            rec = stp.tile([8, NST], F32, tag="recB")
            nc.vector.tensor_scalar_add(rec[:], bs_ps[:], 1e-30)
            nc.vector.reciprocal(rec[:], rec[:])
            rat = stp.tile([8, NST], F32, tag="ratB")
            nc.vector.tensor_mul(rat[:], sums_ps[0:8], rec[:])
            fac = stp.tile([8, NST], F32, tag="facB")
            nc.scalar.activation(fac[:], rat[:], AF.Sqrt)
            den = stp.tile([8, NST], F32, tag="denB")
            nc.vector.tensor_mul(den[:], sums_ps[32:40], fac[:])
            nc.vector.tensor_scalar_add(den[:], den[:], eps_sb[:])
            nc.vector.reciprocal(den[:], den[:])
            s_sb = stp.tile([8, NST], F32R, tag="sB")
            nc.vector.tensor_mul(s_sb[:], fac[:], den[:])

            # phi' = y * s (broadcast s across partitions via K=8 matmul)
            for hh in range(8):
                sb_ps = pst.tile([128, NST], F32, tag="sbtr")
                nc.tensor.matmul(sb_ps[:], lhsT=ind2_sb[:, hh], rhs=s_sb[:],
                                 start=True, stop=True)
                sbc = sbp.tile([128, NST], F32, tag="sbcs")
                if hh % 4 == 3:
                    nc.vector.tensor_copy(sbc[:], sb_ps[:])
                else:
                    nc.scalar.copy(sbc[:], sb_ps[:])
                for mc in range(2):
                    yo = ys[2 * hh + mc]
                    nc.vector.tensor_mul(yo[:], yo.bitcast(F32)[:], sbc[:])

            # outT accumulation: vss (W2 x_s) then numerator via G
            xs_sb = xp.tile([128, KC, NST], F32R, tag="xsB")
            nc.sync.dma_start(
                xs_sb[:], xsT[:, :, nofs:nofs + NST].rearrange("c p n -> p c n"))
            at_sbs = []
            for oc in range(2):
                osl = slice(oc * 128, oc * 128 + 128)
                at_ps = pa.tile([128, NST], F32, tag="mm")
                for fc in range(KC):
                    nc.tensor.matmul(at_ps[:], lhsT=w2_sb[:, fc, osl],
                                     rhs=xs_sb[:, fc],
                                     start=(fc == 0), stop=False)
                for hh in range(8):
                    for mc in range(2):
                        nc.tensor.matmul(
                            at_ps[:], lhsT=g_sb[:, hh, mc, osl],
                            rhs=ys[2 * hh + mc][:],
                            start=False, stop=(hh == 7 and mc == 1))
                at_sb = asb.tile([128, NST], F32R, tag="atB")
                if oc == 0:
                    nc.scalar.copy(at_sb[:], at_ps[:])
                else:
                    nc.vector.tensor_copy(at_sb[:], at_ps[:])
                at_sbs.append(at_sb)

            # transpose to node-major + Lorentz lift
            for sn in range(NST // 128):
                tr_ps = pst.tile([128, 2, 128], F32R, tag="sbtr")
                for oc in range(2):
                    nc.tensor.transpose(
                        tr_ps[:, oc],
                        at_sbs[oc][:, sn * 128:(sn + 1) * 128], ident_sb[:])
                trf = tr_ps.bitcast(F32).rearrange("p a b -> p (a b)")
                o_sb = obp.tile([128, 257], F32, tag="osb")
                nc.vector.tensor_copy(o_sb[:, 1:257], trf)
                scr2 = scp2.tile([128, 256], F32, tag="scr2")
                ssum = stp.tile([128, 1], F32, tag="ssum")
                nc.scalar.activation(scr2[:], trf, AF.Square, accum_out=ssum[:])
                nc.scalar.activation(o_sb[:, 0:1], ssum[:], AF.Sqrt, bias=1.0)
                nc.sync.dma_start(out[nofs + sn * 128: nofs + (sn + 1) * 128, :],
                                  o_sb[:])

        # software pipeline: pres are AR-independent, posts need G/zt (AR)
        nst = NCHUNK // NST
        mid_g(0)
        pre(0)
        pre(1)
        mid_g(1)
        post(0)
        for stx in range(2, nst):
            pre(stx)
            post(stx - 1)
        post(nst - 1)


def _prep_inputs(query_input, source_input, Wq_w, Wq_b, Wk_w, Wk_b, Wv_w, Wv_b,
                 norm_scale, v_map_w, v_map_b, final_w, final_b):
    def pad_xT(x):
        xt = np.zeros((KC * 128, N), np.float32)
        xt[0:257] = x.T
        xt[257] = 1.0
        return xt.reshape(KC, 128, N)

    def pad_w(w_flat, b_flat):
        wt = np.zeros((KC * 128, HD), np.float32)
        wt[0:257] = w_flat.T
        wt[257] = b_flat
        return wt.reshape(KC, 128, HD)

    xq = np.asarray(query_input, np.float32)
    xs = np.asarray(source_input, np.float32)
    xqT = pad_xT(xq)
    xsT = pad_xT(xs)
    xnm = np.zeros((N, KC * 128), np.float32)
    xnm[:, 0:257] = xs
    xnm[:, 257] = 1.0

    wq_h = pad_w(np.asarray(Wq_w).reshape(HD, 257), np.asarray(Wq_b).reshape(HD))
    wk_h = pad_w(np.asarray(Wk_w).reshape(HD, 257), np.asarray(Wk_b).reshape(HD))

    fw = np.asarray(final_w, np.float64)                     # [256, 2048]
    vm = np.asarray(v_map_w, np.float64)
    wv_flat = np.asarray(Wv_w, np.float64).reshape(HD, 257)  # [2048, 257]
    wv_b = np.asarray(Wv_b, np.float64).reshape(HD)

    # W2 = fw @ (vm @ Wv) : [256, 257]; bias = fw @ (vm @ Wv_b + v_map_b) + final_b
    wvm_flat = np.einsum('od,hdi->hoi', vm, np.asarray(Wv_w, np.float64)
                         ).reshape(HD, 257)
    bvm = (np.asarray(Wv_b, np.float64) @ vm.T
           + np.asarray(v_map_b, np.float64)[None, :]).reshape(HD)
    w2m = fw @ wvm_flat                                      # [256, 257]
    b2 = fw @ bvm + np.asarray(final_b, np.float64)          # [256]
    w2_h = np.zeros((KC * 128, D), np.float32)
    w2_h[0:257] = w2m.T.astype(np.float32)
    w2_h[257] = b2.astype(np.float32)
    w2_h = w2_h.reshape(KC, 128, D)

    # WG[f, h, o] = sum_d wv_pad[f, h*256+d] * fw[o, h*256+d]
    wv_pad = np.zeros((258, HD), np.float64)
    wv_pad[0:257] = wv_flat.T
    wv_pad[257] = wv_b
    wg_h = np.einsum('fhd,ohd->fho', wv_pad.reshape(258, H, D),
                     fw.reshape(D, H, D)).astype(np.float32)
    wg_full = np.zeros((KC * 128, H, D), np.float32)
    wg_full[0:258] = wg_h
    wg_full = wg_full.reshape(KC, 128, H, D)

    zt2 = np.zeros((128, 16, 40), np.float32)
    for c in range(16):
        zt2[:, c, c // 2] = 1.0
    ind = np.zeros((128, 8, 8), np.float32)
    for hh in range(8):
        ind[:, hh, hh] = 1.0
    ind2 = np.zeros((8, 8, 128), np.float32)
    for hh in range(8):
        ind2[hh, hh, :] = 1.0

    s = abs(float(np.asarray(norm_scale))) + EPS
    eps_eff = EPS * s * s
    cons = np.full((8, 1), eps_eff, np.float32)

    common = {
        "wq": wq_h, "wk": wk_h, "w2": w2_h, "wg": wg_full,
        "zt2": zt2, "ind": ind, "ind2": ind2,
        "ident": np.eye(128, dtype=np.float32),
        "cons": cons,
    }
    in_maps = []
    for c in range(NCORES):
        m = dict(common)
        m["xqT"] = np.ascontiguousarray(xqT[:, :, c * NCHUNK:(c + 1) * NCHUNK])
        m["xsT"] = np.ascontiguousarray(xsT[:, :, c * NCHUNK:(c + 1) * NCHUNK])
        m["xnm"] = np.ascontiguousarray(xnm[c * NCHUNK:(c + 1) * NCHUNK, :])
        in_maps.append(m)
    return in_maps


def kernel(reps=1, **inputs):
    nc = _build(reps)
    in_maps = _prep_inputs(**inputs)
    res = run_bass_kernel_spmd(nc, in_maps, list(range(NCORES)))
    return np.concatenate([res.results[c]["out"] for c in range(NCORES)], axis=0)


# ---------------------------------------------------------------------------
# Cached-executable runner (used by test.py for accurate HW timing).
# run_bass_kernel_spmd rebuilds its jit on every call, which re-lowers and
# re-uploads everything; for timing we keep one jitted executable per reps
# value and re-invoke it, so repeat calls measure device execution.

def make_cached_runner(reps, inputs, donate=False):
    import jax
    from jax.sharding import Mesh, PartitionSpec, NamedSharding
    from jax.experimental.shard_map import shard_map
    import concourse.bass2jax as b2j

    nc = _build(reps)
    in_maps = _prep_inputs(**inputs)
    b2j.install_neuronx_cc_hook()
    partition_name = nc.partition_id_tensor.name if nc.partition_id_tensor else None
    in_names, out_names, out_avals, zero_shapes = [], [], [], []
    for alloc in nc.m.functions[0].allocations:
        if not isinstance(alloc, mybir.MemoryLocationSet):
            continue
        name = alloc.memorylocations[0].name
        if alloc.kind == "ExternalInput":
            if name != partition_name:
                in_names.append(name)
        elif alloc.kind == "ExternalOutput":
            shape = tuple(alloc.tensor_shape)
            dtype = mybir.dt.np(alloc.dtype)
            out_names.append(name)
            out_avals.append(jax.core.ShapedArray(shape, dtype))
            zero_shapes.append((shape, dtype))
    n_params = len(in_names)
    n_outs = len(out_avals)
    bind_names = list(in_names) + list(out_names)
    if partition_name is not None:
        bind_names.append(partition_name)

    def _bass_body(*args):
        operands = list(args)
        if partition_name is not None:
            operands.append(b2j.partition_id_tensor())
        outs = b2j._bass_exec_p.bind(
            *operands,
            out_avals=tuple(out_avals),
            in_names=tuple(bind_names),
            out_names=tuple(out_names),
            lowering_input_output_aliases=(),
            sim_require_finite=True,
            sim_require_nnan=True,
            nc=nc,
        )
        return tuple(outs)

    devices = jax.devices()[:NCORES]
    mesh = Mesh(np.asarray(devices), ("core",))
    in_specs = (PartitionSpec("core"),) * (n_params + n_outs)
    out_specs = (PartitionSpec("core"),) * n_outs
    donate_idx = tuple(range(n_params, n_params + n_outs)) if donate else ()
    jf = jax.jit(
        shard_map(_bass_body, mesh=mesh, in_specs=in_specs, out_specs=out_specs,
                  check_rep=False),
        donate_argnums=donate_idx, keep_unused=True,
    )
    sharding = NamedSharding(mesh, PartitionSpec("core"))
    per_core = [[np.asarray(m[name]) for name in in_names] for m in in_maps]
    concat_in = [np.concatenate([per_core[c][i] for c in range(NCORES)], axis=0)
                 for i in range(n_params)]
    dev_in = [jax.device_put(a, sharding) for a in concat_in]
    zeros = [jax.device_put(np.zeros((NCORES * s[0], *s[1:]), d), sharding)
             for (s, d) in zero_shapes]
    jax.block_until_ready(dev_in)
    jax.block_until_ready(zeros)

    def run():
        return jf(*dev_in, *zeros)

    def result_np(outs):
        return np.asarray(outs[0]).reshape(NCORES, *zero_shapes[0][0])

    return run, result_np


# revision 33
# speedup vs baseline: 12462.4652x; 2.3436x over previous
"""Trainium2 Bass kernel for hyperbolic linear-attention transformer layer.

Data-parallel over nodes (N=32768) across 8 NeuronCores.

Math (per rep):
  Phase A (source nodes, node-major tiles of 128):
    k = Wk x_pad  (PE, fp32r) ; z = relu(k) (ACT)
    y = z^2 with per-head sums sy (DVE tensor_tensor_reduce)
    sy2 = sum(y^2) per head (ACT Square accum / DVE ttr)
    phi_k = y * sqrt(sy/sy2)   (in-place)
    B += x_pad^T phi_k         (PE, PSUM accumulation over node tiles)
      - x_pad has a trailing 1-column, so row 257 of B is sum_n(phi_k) "sumk"
  AllReduce(B) over the 8 cores  (2.1 MB, Shared output).
  Mid: G[h] = B_h^T WG_h where WG_h = Wv_pad_h fw_h^T (host-precomputed);
    this folds ktv = B^T Wv and the final projection fw into one matrix.
    sumk columns are DMA-staged into the stats lhsT (zt2).
  Phase B (query nodes, feature-major supertiles of 512):
    q = Wq x_pad (PE) ; z = relu(q) ; y = z^2 ; y2 = y^2
    A,C sums via one matmul per 128-feat chunk (lhsT = [ind | sumk]),
    Bsum via matmul vs y2.  fac = sqrt(A/Bsum); s = fac/(C*fac + eps).
    phi' = y * s (broadcast via K=8 matmul)
    outT = sum_h G_h^T phi'_h + W2 x_s  (W2 = fw @ (vmap Wv) folded on host,
      bias row included)
    PE-transpose outT -> node-major, Lorentz lift, DMA out.

All matmuls are fp32r with moving dim >= 256 (full PE rate).
"""

import os
import numpy as np
import concourse.bass as bass
import concourse.tile as tile
from concourse import bacc, mybir
from concourse.bass_utils import run_bass_kernel_spmd

F32 = mybir.dt.float32
F32R = mybir.dt.float32r
AF = mybir.ActivationFunctionType
ALU = mybir.AluOpType

NCORES = 8
N = 32768
NCHUNK = N // NCORES          # 4096 nodes per core
H = 8
D = 256
HD = H * D                    # 2048
KC = 3                        # contraction chunks: 384 = 3*128 (258 used)
EPS = 1e-6
NST = 512                     # phase-B supertile node count

_CACHE = {}


def _build(reps=1):
    if reps in _CACHE:
        return _CACHE[reps]
    nc = bacc.Bacc("TRN2", target_bir_lowering=False, debug=False,
                   num_devices=NCORES)

    xqT = nc.dram_tensor("xqT", [KC, 128, NCHUNK], F32R, kind="ExternalInput").ap()
    xsT = nc.dram_tensor("xsT", [KC, 128, NCHUNK], F32R, kind="ExternalInput").ap()
    xnm = nc.dram_tensor("xnm", [NCHUNK, KC * 128], F32R, kind="ExternalInput").ap()
    wq = nc.dram_tensor("wq", [KC, 128, HD], F32R, kind="ExternalInput").ap()
    wk = nc.dram_tensor("wk", [KC, 128, HD], F32R, kind="ExternalInput").ap()
    w2 = nc.dram_tensor("w2", [KC, 128, D], F32R, kind="ExternalInput").ap()
    wg = nc.dram_tensor("wg", [KC, 128, H, D], F32R, kind="ExternalInput").ap()
    zt2 = nc.dram_tensor("zt2", [128, 16, 40], F32R, kind="ExternalInput").ap()
    ind = nc.dram_tensor("ind", [128, 8, 8], F32R, kind="ExternalInput").ap()
    ind2 = nc.dram_tensor("ind2", [8, 8, 128], F32R, kind="ExternalInput").ap()
    ident = nc.dram_tensor("ident", [128, 128], F32R, kind="ExternalInput").ap()
    cons = nc.dram_tensor("cons", [8, 1], F32, kind="ExternalInput").ap()
    out = nc.dram_tensor("out", [NCHUNK, 257], F32, kind="ExternalOutput").ap()

    with tile.TileContext(nc) as tc:
        _body(nc, tc, reps, xqT, xsT, xnm, wq, wk, w2, wg, zt2, ind, ind2,
              ident, cons, out)
    nc.compile()
    _CACHE[reps] = nc
    return nc


def _body(nc, tc, reps, xqT, xsT, xnm, wq, wk, w2, wg, zt2, ind, ind2,
          ident, cons, out):
    import contextlib
    stack = contextlib.ExitStack()
    with stack:
        cpool = stack.enter_context(tc.tile_pool(name="const", bufs=1))
        dpool = stack.enter_context(tc.tile_pool(name="dram", bufs=1, space="DRAM"))

        ind_sb = cpool.tile([128, 8, 8], F32R)
        nc.sync.dma_start(ind_sb[:], ind[:])
        ind2_sb = cpool.tile([8, 8, 128], F32R)
        nc.sync.dma_start(ind2_sb[:], ind2[:])
        ident_sb = cpool.tile([128, 128], F32R)
        nc.sync.dma_start(ident_sb[:], ident[:])
        eps_sb = cpool.tile([8, 1], F32)
        nc.sync.dma_start(eps_sb[:], cons[:])
        zt_sb = cpool.tile([128, 16, 40], F32R)
        nc.sync.dma_start(zt_sb[:], zt2[:])

        for rep in range(reps):
            ar_in = [dpool.tile([258, 1024], F32, tag=f"ari{rep}g{g}",
                                name=f"ari{rep}g{g}") for g in range(2)]
            ar_out = [dpool.tile([258, 1024], F32,
                                 tag=f"aro{rep}g{g}", name=f"aro{rep}g{g}")
                      for g in range(2)]
            if not os.environ.get("KT_SKIP_A"):
                _phase_a(nc, tc, xsT, xnm, wk, ar_in, ar_out)
            if not os.environ.get("KT_SKIP_B"):
                _phase_b(nc, tc, xqT, xsT, wq, w2, wg, zt_sb, ind_sb, ind2_sb,
                         ident_sb, eps_sb, ar_out, out)
            else:
                with tc.tile_pool(name="oBtmp", bufs=1) as ob:
                    o_sb = ob.tile([128, 257], F32)
                    nc.sync.dma_start(o_sb[:], ar_out[0][0:128, 0:257])
                    for t0_ in range(NCHUNK // 128):
                        nc.sync.dma_start(out[t0_ * 128:(t0_ + 1) * 128, :], o_sb[:])


def _phase_a(nc, tc, xsT, xnm, wk, ar_in, ar_out):
    import contextlib
    with contextlib.ExitStack() as st:
        wpool = st.enter_context(tc.tile_pool(name="wA", bufs=1))
        xp = st.enter_context(tc.tile_pool(name="xA", bufs=3))
        xnp_ = st.enter_context(tc.tile_pool(name="xnA", bufs=4))
        yp = st.enter_context(tc.tile_pool(name="yA", bufs=3))
        zap = st.enter_context(tc.tile_pool(name="zA", bufs=2))
        scp = st.enter_context(tc.tile_pool(name="scA", bufs=2))
        stp = st.enter_context(tc.tile_pool(name="stA", bufs=4))
        drp = st.enter_context(tc.tile_pool(name="drA", bufs=2))
        pk = st.enter_context(tc.tile_pool(name="psAk", bufs=2, space="PSUM"))
        pb = st.enter_context(tc.tile_pool(name="psAb", bufs=1, space="PSUM"))

        wk_sb = wpool.tile([128, KC, HD], F32R)
        nc.sync.dma_start(wk_sb[:], wk.rearrange("c p n -> p c n"))

        ntiles = NCHUNK // 128
        for g in range(2):
            gofs = g * 1024
            b_ps0 = pb.tile([128, 1024], F32, tag="bps0")
            b_ps1 = pb.tile([128, 1024], F32, tag="bps1")
            b_ps2 = pb.tile([2, 1024], F32, tag="bps2")
            b_tiles = (b_ps0, b_ps1, b_ps2)
            prev = None
            for t in range(ntiles):
                xs_sb = xp.tile([128, KC, 128], F32R, tag="xs")
                nc.sync.dma_start(
                    xs_sb[:],
                    xsT[:, :, t * 128:(t + 1) * 128].rearrange("c p n -> p c n"))
                xn_sb = xnp_.tile([128, KC, 128], F32R, tag="xn")
                nc.sync.dma_start(
                    xn_sb[:],
                    xnm[t * 128:(t + 1) * 128, :].rearrange("n (c f) -> n c f",
                                                            c=KC))

                y = yp.tile([128, 1024], F32R, tag="y")
                yf = y.bitcast(F32)
                z = zap.tile([128, 1024], F32, tag="zA")
                sy = stp.tile([128, 4], F32, tag="sy")
                sy2 = stp.tile([128, 4], F32, tag="sy2")
                for blk in range(2):
                    kp = pk.tile([128, 512], F32, tag="kp")
                    for c in range(KC):
                        nc.tensor.matmul(
                            kp[:], lhsT=xs_sb[:, c],
                            rhs=wk_sb[:, c, gofs + blk * 512: gofs + blk * 512 + 512],
                            start=(c == 0), stop=(c == KC - 1))
                    nc.vector.tensor_scalar_max(
                        z[:, blk * 512:(blk + 1) * 512], kp[:], 0.0)
                # y = z^2 with per-head accumulated sums sy (ACT Square+accum)
                for hh in range(4):
                    sl = slice(hh * 256, hh * 256 + 256)
                    nc.scalar.activation(y[:, sl], z[:, sl], AF.Square,
                                         accum_out=sy[:, hh:hh + 1])
                # sy2 = sum(y^2) per head (ACT Square with accum)
                for hh in range(4):
                    sl = slice(hh * 256, hh * 256 + 256)
                    scr = scp.tile([128, 256], F32, tag="scr")
                    nc.scalar.activation(scr[:], yf[:, sl], AF.Square,
                                         accum_out=sy2[:, hh:hh + 1])
                rec = stp.tile([128, 4], F32, tag="rec")
                nc.vector.reciprocal(rec[:], sy2[:])
                rat = stp.tile([128, 4], F32, tag="rat")
                nc.vector.tensor_mul(rat[:], sy[:], rec[:])
                fac = stp.tile([128, 4], F32, tag="fac")
                nc.scalar.activation(fac[:], rat[:], AF.Sqrt)
                # phi = y * fac, in place (2 on DVE, 2 on ACT)
                for hh in range(4):
                    sl = slice(hh * 256, hh * 256 + 256)
                    if hh < 2:
                        nc.vector.tensor_scalar_mul(y[:, sl], yf[:, sl],
                                                    fac[:, hh:hh + 1])
                    else:
                        nc.scalar.activation(y[:, sl], yf[:, sl], AF.Copy,
                                             scale=fac[:, hh:hh + 1])

                # B accumulation for the PREVIOUS tile (software pipeline:
                # keeps PE busy with tile t's projection while tile t-1's
                # phi chain drains on DVE/ACT)
                if t > 0:
                    _b_accum(nc, b_tiles, prev[0], prev[1], t - 1, ntiles)
                prev = (xn_sb, y)
            _b_accum(nc, b_tiles, prev[0], prev[1], ntiles - 1, ntiles)

            d0 = drp.tile([128, 1024], F32, tag="d0")
            nc.vector.tensor_copy(d0[:], b_ps0[:])
            nc.sync.dma_start(ar_in[g][0:128, :], d0[:])
            d1 = drp.tile([128, 1024], F32, tag="d1")
            nc.scalar.copy(d1[:], b_ps1[:])
            nc.sync.dma_start(ar_in[g][128:256, :], d1[:])
            d2 = drp.tile([2, 1024], F32, tag="d2")
            nc.vector.tensor_copy(d2[:], b_ps2[:])
            nc.sync.dma_start(ar_in[g][256:258, :], d2[:])
            # per-group AllReduce: group 0's AR overlaps group 1's compute
            if os.environ.get("KT_LOCAL_AR"):
                nc.sync.dma_start(ar_out[g][:], ar_in[g][:])
            else:
                nc.gpsimd.collective_compute(
                    "AllReduce", ALU.add,
                    replica_groups=[list(range(NCORES))],
                    ins=[ar_in[g].opt()], outs=[ar_out[g].opt()])


def _b_accum(nc, b_tiles, xn_sb, y, t, ntiles):
    b_ps0, b_ps1, b_ps2 = b_tiles
    for blk in range(2):
        ms = slice(blk * 512, blk * 512 + 512)
        nc.tensor.matmul(b_ps0[:, ms], lhsT=xn_sb[:, 0], rhs=y[:, ms],
                         start=(t == 0), stop=(t == ntiles - 1))
        nc.tensor.matmul(b_ps1[:, ms], lhsT=xn_sb[:, 1], rhs=y[:, ms],
                         start=(t == 0), stop=(t == ntiles - 1))
        nc.tensor.matmul(b_ps2[:, ms], lhsT=xn_sb[:, 2, 0:2], rhs=y[:, ms],
                         start=(t == 0), stop=(t == ntiles - 1))


def _phase_b(nc, tc, xqT, xsT, wq, w2, wg, zt_sb, ind_sb, ind2_sb,
             ident_sb, eps_sb, ar_out, out):
    import contextlib
    with contextlib.ExitStack() as st:
        wpool = st.enter_context(tc.tile_pool(name="wB", bufs=1))
        mpool = st.enter_context(tc.tile_pool(name="midB", bufs=1))
        xp = st.enter_context(tc.tile_pool(name="xB", bufs=2))
        yp = st.enter_context(tc.tile_pool(name="yB", bufs=30))
        zbp = st.enter_context(tc.tile_pool(name="zbB", bufs=3))
        y2p = st.enter_context(tc.tile_pool(name="y2B", bufs=3))
        scp2 = st.enter_context(tc.tile_pool(name="sc2B", bufs=2))
        stp = st.enter_context(tc.tile_pool(name="stB", bufs=1))
        sbp = st.enter_context(tc.tile_pool(name="sbB", bufs=2))
        asb = st.enter_context(tc.tile_pool(name="aB", bufs=3))
        obp = st.enter_context(tc.tile_pool(name="oB", bufs=3))
        ps = st.enter_context(tc.tile_pool(name="psBs", bufs=2, space="PSUM"))
        pbs = st.enter_context(tc.tile_pool(name="psBbs", bufs=2, space="PSUM"))
        pa = st.enter_context(tc.tile_pool(name="psBa", bufs=2, space="PSUM"))
        pst = st.enter_context(tc.tile_pool(name="psBt", bufs=2, space="PSUM"))

        wq_sb = wpool.tile([128, KC, HD], F32R)
        nc.sync.dma_start(wq_sb[:], wq.rearrange("c p n -> p c n"))
        w2_sb = wpool.tile([128, KC, D], F32R)
        nc.sync.dma_start(w2_sb[:], w2.rearrange("c p n -> p c n"))
        g_sb = wpool.tile([128, H, 2, D], F32R)

        def mid_g(g):
            # G = B^T WG for this half's heads; stage sumk cols into zt
            bf0 = mpool.tile([128, 1024], F32R, tag="bf0")
            nc.sync.dma_start(bf0.bitcast(F32)[:], ar_out[g][0:128, :])
            bf1 = mpool.tile([128, 1024], F32R, tag="bf1")
            nc.sync.dma_start(bf1.bitcast(F32)[:], ar_out[g][128:256, :])
            bf2 = mpool.tile([2, 1024], F32R, tag="bf2")
            nc.sync.dma_start(bf2.bitcast(F32)[:], ar_out[g][256:258, :])
            wg_sb = mpool.tile([128, KC, 4, D], F32R, tag="wgh")
            nc.sync.dma_start(
                wg_sb[:], wg[:, :, g * 4:(g + 1) * 4].rearrange(
                    "c p h n -> p c h n"))
            bfs = [bf0, bf1, bf2]
            for hl in range(4):
                hh = g * 4 + hl
                for mc in range(2):
                    msl = slice(hl * 256 + mc * 128, hl * 256 + mc * 128 + 128)
                    gp = pa.tile([128, NST], F32, tag="mm")
                    for fc in range(KC):
                        lhs = bfs[fc][:, msl] if fc < 2 else bfs[2][0:2, msl]
                        rhs = (wg_sb[:, fc, hl] if fc < 2
                               else wg_sb[0:2, fc, hl])
                        nc.tensor.matmul(gp[:, 0:D], lhsT=lhs, rhs=rhs,
                                         start=(fc == 0), stop=(fc == KC - 1))
                    if (hl * 2 + mc) % 2 == 0:
                        nc.vector.tensor_copy(g_sb[:, hh, mc], gp[:, 0:D])
                    else:
                        nc.scalar.copy(g_sb[:, hh, mc], gp[:, 0:D])
            for cl in range(8):
                c = g * 8 + cl
                hh = c // 2
                nc.gpsimd.dma_start(
                    zt_sb[:, c, 32 + hh:33 + hh],
                    ar_out[g][257:258, cl * 128:(cl + 1) * 128].rearrange(
                        "r (p o) -> (r p) o", o=1))

        state = {}

        def pre(stx):
            nofs = stx * NST
            xq_sb = xp.tile([128, KC, NST], F32R, tag="xq")
            nc.sync.dma_start(
                xq_sb[:], xqT[:, :, nofs:nofs + NST].rearrange("c p n -> p c n"))
            sums_ps = ps.tile([64, NST], F32, tag="sums")
            bs_ps = pbs.tile([8, NST], F32, tag="bsum")
            ys = []
            for c in range(16):
                qp = pa.tile([128, NST], F32, tag="mm")
                for kc in range(KC):
                    nc.tensor.matmul(
                        qp[:], lhsT=wq_sb[:, kc, c * 128:(c + 1) * 128],
                        rhs=xq_sb[:, kc], start=(kc == 0), stop=(kc == KC - 1))
                zb = zbp.tile([128, NST], F32, tag="zbB")
                nc.vector.tensor_scalar_max(zb[:], qp[:], 0.0)
                y = yp.tile([128, NST], F32R, tag="yB")
                yf = y.bitcast(F32)
                nc.scalar.activation(y[:], zb[:], AF.Square)
                y2 = y2p.tile([128, NST], F32R, tag="y2B")
                nc.scalar.activation(y2[:], yf[:], AF.Square)
                nc.tensor.matmul(bs_ps[:], lhsT=ind_sb[:, c // 2],
                                 rhs=y2[:], start=(c == 0), stop=(c == 15))
                ys.append(y)
            state[stx] = (ys, sums_ps, bs_ps)

        def post(stx):
            nofs = stx * NST
            ys, sums_ps, bs_ps = state.pop(stx)
            # A,C = per-head sums of y and sumk-weighted sums of y
            # (lhsT cols 0..7 = head indicators, 8..15 = sumk staged from AR)
            for c in range(16):
                nc.tensor.matmul(sums_ps[0:40], lhsT=zt_sb[:, c], rhs=ys[c][:],
                                 start=(c == 0), stop=(c == 15))
            # stats: fac = sqrt(A/Bsum); s = fac / (C*fac + eps)
            rec = stp.tile([8, NST], F32, tag="recB")
            nc.vector.tensor_scalar_add(rec[:], bs_ps[:], 1e-30)
            nc.vector.reciprocal(rec[:], rec[:])
            rat = stp.tile([8, NST], F32, tag="ratB")
            nc.vector.tensor_mul(rat[:], sums_ps[0:8], rec[:])
            fac = stp.tile([8, NST], F32, tag="facB")
            nc.scalar.activation(fac[:], rat[:], AF.Sqrt)
            den = stp.tile([8, NST], F32, tag="denB")
            nc.vector.tensor_mul(den[:], sums_ps[32:40], fac[:])
            nc.vector.tensor_scalar_add(den[:], den[:], eps_sb[:])
            nc.vector.reciprocal(den[:], den[:])
            s_sb = stp.tile([8, NST], F32R, tag="sB")
            nc.vector.tensor_mul(s_sb[:], fac[:], den[:])

            # phi' = y * s (broadcast s across partitions via K=8 matmul)
            for hh in range(8):
                sb_ps = pst.tile([128, NST], F32, tag="sbtr")
                nc.tensor.matmul(sb_ps[:], lhsT=ind2_sb[:, hh], rhs=s_sb[:],
                                 start=True, stop=True)
                sbc = sbp.tile([128, NST], F32, tag="sbcs")
                if hh % 4 == 3:
                    nc.vector.tensor_copy(sbc[:], sb_ps[:])
                else:
                    nc.scalar.copy(sbc[:], sb_ps[:])
                for mc in range(2):
                    yo = ys[2 * hh + mc]
                    nc.vector.tensor_mul(yo[:], yo.bitcast(F32)[:], sbc[:])

            # outT accumulation: vss (W2 x_s) then numerator via G
            xs_sb = xp.tile([128, KC, NST], F32R, tag="xsB")
            nc.sync.dma_start(
                xs_sb[:], xsT[:, :, nofs:nofs + NST].rearrange("c p n -> p c n"))
            at_sbs = []
            for oc in range(2):
                osl = slice(oc * 128, oc * 128 + 128)
                at_ps = pa.tile([128, NST], F32, tag="mm")
                for fc in range(KC):
                    nc.tensor.matmul(at_ps[:], lhsT=w2_sb[:, fc, osl],
                                     rhs=xs_sb[:, fc],
                                     start=(fc == 0), stop=False)
                for hh in range(8):
                    for mc in range(2):
                        nc.tensor.matmul(
                            at_ps[:], lhsT=g_sb[:, hh, mc, osl],
                            rhs=ys[2 * hh + mc][:],
                            start=False, stop=(hh == 7 and mc == 1))
                at_sb = asb.tile([128, NST], F32R, tag="atB")
                if oc == 0:
                    nc.scalar.copy(at_sb[:], at_ps[:])
                else:
                    nc.vector.tensor_copy(at_sb[:], at_ps[:])
                at_sbs.append(at_sb)

            # transpose to node-major + Lorentz lift
            for sn in range(NST // 128):
                tr_ps = pst.tile([128, 2, 128], F32R, tag="sbtr")
                for oc in range(2):
                    nc.tensor.transpose(
                        tr_ps[:, oc],
                        at_sbs[oc][:, sn * 128:(sn + 1) * 128], ident_sb[:])
                trf = tr_ps.bitcast(F32).rearrange("p a b -> p (a b)")
                o_sb = obp.tile([128, 257], F32, tag="osb")
                nc.vector.tensor_copy(o_sb[:, 1:257], trf)
                scr2 = scp2.tile([128, 256], F32, tag="scr2")
                ssum = stp.tile([128, 1], F32, tag="ssum")
                nc.scalar.activation(scr2[:], trf, AF.Square, accum_out=ssum[:])
                nc.scalar.activation(o_sb[:, 0:1], ssum[:], AF.Sqrt, bias=1.0)
                nc.sync.dma_start(out[nofs + sn * 128: nofs + (sn + 1) * 128, :],
                                  o_sb[:])

        # software pipeline: pres are AR-independent, posts need G/zt (AR)
        nst = NCHUNK // NST
        mid_g(0)
        pre(0)
        pre(1)
        mid_g(1)
        post(0)
        for stx in range(2, nst):
            pre(stx)
            post(stx - 1)
        post(nst - 1)


def _prep_inputs(query_input, source_input, Wq_w, Wq_b, Wk_w, Wk_b, Wv_w, Wv_b,
                 norm_scale, v_map_w, v_map_b, final_w, final_b):
    def pad_xT(x):
        xt = np.zeros((KC * 128, N), np.float32)
        xt[0:257] = x.T
        xt[257] = 1.0
        return xt.reshape(KC, 128, N)

    def pad_w(w_flat, b_flat):
        wt = np.zeros((KC * 128, HD), np.float32)
        wt[0:257] = w_flat.T
        wt[257] = b_flat
        return wt.reshape(KC, 128, HD)

    xq = np.asarray(query_input, np.float32)
    xs = np.asarray(source_input, np.float32)
    xqT = pad_xT(xq)
    xsT = pad_xT(xs)
    xnm = np.zeros((N, KC * 128), np.float32)
    xnm[:, 0:257] = xs
    xnm[:, 257] = 1.0

    wq_h = pad_w(np.asarray(Wq_w).reshape(HD, 257), np.asarray(Wq_b).reshape(HD))
    wk_h = pad_w(np.asarray(Wk_w).reshape(HD, 257), np.asarray(Wk_b).reshape(HD))

    fw = np.asarray(final_w, np.float64)                     # [256, 2048]
    vm = np.asarray(v_map_w, np.float64)
    wv_flat = np.asarray(Wv_w, np.float64).reshape(HD, 257)  # [2048, 257]
    wv_b = np.asarray(Wv_b, np.float64).reshape(HD)

    # W2 = fw @ (vm @ Wv) : [256, 257]; bias = fw @ (vm @ Wv_b + v_map_b) + final_b
    wvm_flat = np.einsum('od,hdi->hoi', vm, np.asarray(Wv_w, np.float64)
                         ).reshape(HD, 257)
    bvm = (np.asarray(Wv_b, np.float64) @ vm.T
           + np.asarray(v_map_b, np.float64)[None, :]).reshape(HD)
    w2m = fw @ wvm_flat                                      # [256, 257]
    b2 = fw @ bvm + np.asarray(final_b, np.float64)          # [256]
    w2_h = np.zeros((KC * 128, D), np.float32)
    w2_h[0:257] = w2m.T.astype(np.float32)
    w2_h[257] = b2.astype(np.float32)
    w2_h = w2_h.reshape(KC, 128, D)

    # WG[f, h, o] = sum_d wv_pad[f, h*256+d] * fw[o, h*256+d]
    wv_pad = np.zeros((258, HD), np.float64)
    wv_pad[0:257] = wv_flat.T
    wv_pad[257] = wv_b
    wg_h = np.einsum('fhd,ohd->fho', wv_pad.reshape(258, H, D),
                     fw.reshape(D, H, D)).astype(np.float32)
    wg_full = np.zeros((KC * 128, H, D), np.float32)
    wg_full[0:258] = wg_h
    wg_full = wg_full.reshape(KC, 128, H, D)

    zt2 = np.zeros((128, 16, 40), np.float32)
    for c in range(16):
        zt2[:, c, c // 2] = 1.0
    ind = np.zeros((128, 8, 8), np.float32)
    for hh in range(8):
        ind[:, hh, hh] = 1.0
    ind2 = np.zeros((8, 8, 128), np.float32)
    for hh in range(8):
        ind2[hh, hh, :] = 1.0

    s = abs(float(np.asarray(norm_scale))) + EPS
    eps_eff = EPS * s * s
    cons = np.full((8, 1), eps_eff, np.float32)

    common = {
        "wq": wq_h, "wk": wk_h, "w2": w2_h, "wg": wg_full,
        "zt2": zt2, "ind": ind, "ind2": ind2,
        "ident": np.eye(128, dtype=np.float32),
        "cons": cons,
    }
    in_maps = []
    for c in range(NCORES):
        m = dict(common)
        m["xqT"] = np.ascontiguousarray(xqT[:, :, c * NCHUNK:(c + 1) * NCHUNK])
        m["xsT"] = np.ascontiguousarray(xsT[:, :, c * NCHUNK:(c + 1) * NCHUNK])
        m["xnm"] = np.ascontiguousarray(xnm[c * NCHUNK:(c + 1) * NCHUNK, :])
        in_maps.append(m)
    return in_maps


def kernel(reps=1, **inputs):
    nc = _build(reps)
    in_maps = _prep_inputs(**inputs)
    res = run_bass_kernel_spmd(nc, in_maps, list(range(NCORES)))
    return np.concatenate([res.results[c]["out"] for c in range(NCORES)], axis=0)


# ---------------------------------------------------------------------------
# Cached-executable runner (used by test.py for accurate HW timing).
# run_bass_kernel_spmd rebuilds its jit on every call, which re-lowers and
# re-uploads everything; for timing we keep one jitted executable per reps
# value and re-invoke it, so repeat calls measure device execution.

def make_cached_runner(reps, inputs, donate=False):
    import jax
    from jax.sharding import Mesh, PartitionSpec, NamedSharding
    from jax.experimental.shard_map import shard_map
    import concourse.bass2jax as b2j

    nc = _build(reps)
    in_maps = _prep_inputs(**inputs)
    b2j.install_neuronx_cc_hook()
    partition_name = nc.partition_id_tensor.name if nc.partition_id_tensor else None
    in_names, out_names, out_avals, zero_shapes = [], [], [], []
    for alloc in nc.m.functions[0].allocations:
        if not isinstance(alloc, mybir.MemoryLocationSet):
            continue
        name = alloc.memorylocations[0].name
        if alloc.kind == "ExternalInput":
            if name != partition_name:
                in_names.append(name)
        elif alloc.kind == "ExternalOutput":
            shape = tuple(alloc.tensor_shape)
            dtype = mybir.dt.np(alloc.dtype)
            out_names.append(name)
            out_avals.append(jax.core.ShapedArray(shape, dtype))
            zero_shapes.append((shape, dtype))
    n_params = len(in_names)
    n_outs = len(out_avals)
    bind_names = list(in_names) + list(out_names)
    if partition_name is not None:
        bind_names.append(partition_name)

    def _bass_body(*args):
        operands = list(args)
        if partition_name is not None:
            operands.append(b2j.partition_id_tensor())
        outs = b2j._bass_exec_p.bind(
            *operands,
            out_avals=tuple(out_avals),
            in_names=tuple(bind_names),
            out_names=tuple(out_names),
            lowering_input_output_aliases=(),
            sim_require_finite=True,
            sim_require_nnan=True,
            nc=nc,
        )
        return tuple(outs)

    devices = jax.devices()[:NCORES]
    mesh = Mesh(np.asarray(devices), ("core",))
    in_specs = (PartitionSpec("core"),) * (n_params + n_outs)
    out_specs = (PartitionSpec("core"),) * n_outs
    donate_idx = tuple(range(n_params, n_params + n_outs)) if donate else ()
    jf = jax.jit(
        shard_map(_bass_body, mesh=mesh, in_specs=in_specs, out_specs=out_specs,
                  check_rep=False),
        donate_argnums=donate_idx, keep_unused=True,
    )
    sharding = NamedSharding(mesh, PartitionSpec("core"))
    per_core = [[np.asarray(m[name]) for name in in_names] for m in in_maps]
    concat_in = [np.concatenate([per_core[c][i] for c in range(NCORES)], axis=0)
                 for i in range(n_params)]
    dev_in = [jax.device_put(a, sharding) for a in concat_in]
    zeros = [jax.device_put(np.zeros((NCORES * s[0], *s[1:]), d), sharding)
             for (s, d) in zero_shapes]
    jax.block_until_ready(dev_in)
    jax.block_until_ready(zeros)

    def run():
        return jf(*dev_in, *zeros)

    def result_np(outs):
        return np.asarray(outs[0]).reshape(NCORES, *zero_shapes[0][0])

    return run, result_np
